# revision 15
# baseline (speedup 1.0000x reference)
"""Trainium2 Bass kernel for nn_ConformerMoEEncoderLayer.

Sharding: data-parallel, 1 batch element per NeuronCore (B=8, 8 cores).
Expert selection (dialectid) is resolved on the host; each core receives only
its chosen expert's FF weights (the reference's gate softmax output is unused).

Host-side folding:
  - every pre-norm LN gamma/beta folded into the following matmul weights
  - 1/sqrt(dk) folded into wq/bq; ff_scale=0.5 folded into w2/b2 of both FFs
  - BatchNorm (eval) folded into the depthwise-conv weights/bias

Device program per core (residual stream x kept [T,D], T on partitions):
  - LN via bn_stats/bn_aggr; block input transposed to [D,T] with PE transpose
  - FF chains: h=[H,T] via lhsT=W1, then back to [T,D] via lhsT=h
  - attention: transposed scores (lhsT=k, rhs=q), exp without max-subtraction
    (scores are small; mask bias folded into the exp activation when present),
    softmax denominator via a ones-column appended to v, per-head recip
    broadcast with a K=1 matmul from partition 64
  - depthwise conv31 as 31 accumulating diagonal-lhsT matmuls over shifted
    slices of the padded GLU output

Matmuls run in float32r (1 cycle/row vs 4 for fp32; HW truncates inputs to 11
explicit mantissa bits).
"""

import numpy as np

import concourse.bass as bass
import concourse.bacc as bacc
import concourse.mybir as mybir
import concourse.tile as tile
from concourse.bass_utils import run_bass_kernel_spmd
from concourse.masks import make_identity

B, T, D, H, NH, E, KTAPS = 8, 1024, 512, 2048, 8, 8, 31
DK = D // NH                 # 64
P = 128
TT = T // P                  # 8 t-tiles
DP = D // P                  # 4 d-tiles
HP = H // P                  # 16 h-tiles
NCORES = 8
LN_EPS = 1e-12
BN_EPS = 1e-5
PAD = KTAPS // 2             # 15

F32 = mybir.dt.float32
F32R = mybir.dt.float32r
AF = mybir.ActivationFunctionType
OP = mybir.AluOpType

_cache = {}
last_result = None  # BassKernelResults of the most recent kernel() call


# ---------------------------------------------------------------- host prep

def _f32(a):
    return np.ascontiguousarray(np.asarray(a), dtype=np.float32)


def _fold_weights(inp):
    g_ffmac, b_ffmac = _f32(inp["ln_ffmac_g"]), _f32(inp["ln_ffmac_b"])
    w1_mac, b1_mac = _f32(inp["w1_mac"]), _f32(inp["b1_mac"])
    w1m = g_ffmac[:, None] * w1_mac
    b1m = b_ffmac @ w1_mac + b1_mac
    w2m = 0.5 * _f32(inp["w2_mac"])
    b2m = 0.5 * _f32(inp["b2_mac"])

    g_mha, b_mha = _f32(inp["ln_mha_g"]), _f32(inp["ln_mha_b"])
    scale = np.float32(1.0 / np.sqrt(np.float32(DK)))
    wq = g_mha[:, None] * _f32(inp["wq"]) * scale
    bq = (b_mha @ _f32(inp["wq"]) + _f32(inp["bq"])) * scale
    wk = g_mha[:, None] * _f32(inp["wk"])
    bk = b_mha @ _f32(inp["wk"]) + _f32(inp["bk"])
    wv = g_mha[:, None] * _f32(inp["wv"])
    bv = b_mha @ _f32(inp["wv"]) + _f32(inp["bv"])
    # odd heads of o land at partitions 64-127 via a swap of 64-blocks
    wo, bo = _f32(inp["wo"]), _f32(inp["bo"])

    g_conv, b_conv = _f32(inp["ln_conv_g"]), _f32(inp["ln_conv_b"])
    pw1 = g_conv[:, None] * _f32(inp["pw1_w"])
    pw1b = b_conv @ _f32(inp["pw1_w"]) + _f32(inp["pw1_b"])
    s = _f32(inp["bn_g"]) / np.sqrt(_f32(inp["bn_var"]) + np.float32(BN_EPS))
    dww = s[:, None] * _f32(inp["dw_w"])[:, 0, :]          # [D, 31]
    dwb = s * (_f32(inp["dw_b"]) - _f32(inp["bn_mean"])) + _f32(inp["bn_b"])
    pw2, pw2b = _f32(inp["pw2_w"]), _f32(inp["pw2_b"])

    g_ff, b_ff = _f32(inp["ln_ff_g"]), _f32(inp["ln_ff_b"])
    e_w1, e_b1 = _f32(inp["e_w1"]), _f32(inp["e_b1"])
    e_w2, e_b2 = _f32(inp["e_w2"]), _f32(inp["e_b2"])
    chosen = np.clip(np.asarray(inp["dialectid"]).astype(np.int64) - 1, 0, E - 1)
    we1 = np.stack([g_ff[:, None] * e_w1[c] for c in chosen])       # [B, D, H]
    be1 = np.stack([b_ff @ e_w1[c] + e_b1[c] for c in chosen])      # [B, H]
    we2 = np.stack([0.5 * e_w2[c] for c in chosen])                 # [B, H, D]
    be2 = np.stack([0.5 * e_b2[c] for c in chosen])                 # [B, D]

    mask = np.asarray(inp["mask"]).reshape(B, T)
    maskbias = np.where(mask, np.float32(0.0), np.float32(-30.0)).astype(np.float32)

    return dict(
        w1m=_f32(w1m), b1m=_f32(b1m), w2m=_f32(w2m), b2m=_f32(b2m),
        wq=_f32(wq), bq=_f32(bq), wk=_f32(wk), bk=_f32(bk),
        wv=_f32(wv), bv=_f32(bv), wo=wo, bo=bo,
        pw1=_f32(pw1), pw1b=_f32(pw1b), dww=_f32(dww), dwb=_f32(dwb),
        pw2=pw2, pw2b=pw2b,
        we1=_f32(we1), be1=_f32(be1), we2=_f32(we2), be2=_f32(be2),
        g_fin=_f32(inp["ln_final_g"]), b_fin=_f32(inp["ln_final_b"]),
        maskbias=maskbias, x=_f32(inp["x"]),
    )


def _flags(fw):
    nz = lambda a: bool(np.any(a != 0))
    return dict(
        b1m=nz(fw["b1m"]), b2m=nz(fw["b2m"]),
        bq=nz(fw["bq"]), bk=nz(fw["bk"]), bv=nz(fw["bv"]), bo=nz(fw["bo"]),
        pw1b=nz(fw["pw1b"]), dwb=nz(fw["dwb"]), pw2b=nz(fw["pw2b"]),
        be1=nz(fw["be1"]), be2=nz(fw["be2"]),
        mask=nz(fw["maskbias"]),
        fin=bool(np.any(fw["g_fin"] != 1.0) or np.any(fw["b_fin"] != 0.0)),
    )


# ------------------------------------------------------------- device build

def _build(fl):
    nc = bacc.Bacc("TRN2")

    dt_ = dict(
        x_d=nc.dram_tensor("x", [T, D], F32, kind="ExternalInput"),
        w1m_d=nc.dram_tensor("w1m", [D, H], F32R, kind="ExternalInput"),
        w2m_d=nc.dram_tensor("w2m", [H, D], F32R, kind="ExternalInput"),
        wq_d=nc.dram_tensor("wq", [D, D], F32R, kind="ExternalInput"),
        wk_d=nc.dram_tensor("wk", [D, D], F32R, kind="ExternalInput"),
        wv_d=nc.dram_tensor("wv", [D, D], F32R, kind="ExternalInput"),
        wo_d=nc.dram_tensor("wo", [D, D], F32R, kind="ExternalInput"),
        pw1_d=nc.dram_tensor("pw1", [D, 2 * D], F32R, kind="ExternalInput"),
        pw2_d=nc.dram_tensor("pw2", [D, D], F32R, kind="ExternalInput"),
        dww_d=nc.dram_tensor("dww", [D, KTAPS], F32, kind="ExternalInput"),
        we1_d=nc.dram_tensor("we1", [D, H], F32R, kind="ExternalInput"),
        we2_d=nc.dram_tensor("we2", [H, D], F32R, kind="ExternalInput"),
        cst_d=nc.dram_tensor("cst", [2, P], F32R, kind="ExternalInput"),
        y_d=nc.dram_tensor("y", [T, D], F32, kind="ExternalOutput"),
    )
    bias_d = {}
    for nm, n in [("b1m", H), ("bq", D), ("bk", D), ("bv", D),
                  ("pw1b", 2 * D), ("dwb", D), ("be1", H)]:
        if fl[nm]:
            bias_d[nm] = nc.dram_tensor(nm, [n], F32, kind="ExternalInput")
    for nm in ("b2m", "bo", "pw2b", "be2"):
        if fl[nm]:
            bias_d[nm] = nc.dram_tensor(nm, [1, D], F32, kind="ExternalInput")
    if fl["mask"]:
        bias_d["maskbias"] = nc.dram_tensor("maskbias", [T], F32,
                                            kind="ExternalInput")
    if fl["fin"]:
        bias_d["g_fin"] = nc.dram_tensor("g_fin", [D], F32, kind="ExternalInput")
        bias_d["b_fin"] = nc.dram_tensor("b_fin", [D], F32, kind="ExternalInput")
    dt_["bias_d"] = bias_d

    with tile.TileContext(nc) as tc:
        _emit(nc, tc, fl, dt_)
    nc.compile()
    return nc


def _emit(nc, tc, fl, dt_):
    from contextlib import ExitStack

    bias_d = dt_["bias_d"]
    with ExitStack() as ctx:
        # ------------- persistent pools
        pconst = ctx.enter_context(tc.tile_pool(name="pconst", bufs=1))
        px = ctx.enter_context(tc.tile_pool(name="px", bufs=1))
        pstat = ctx.enter_context(tc.tile_pool(name="pstat", bufs=4))
        pxlnb = ctx.enter_context(tc.tile_pool(name="pxlnb", bufs=4))
        pxlna = ctx.enter_context(tc.tile_pool(name="pxlna", bufs=1))
        ps_tp = ctx.enter_context(tc.tile_pool(name="ps_tp", bufs=2,
                                               space="PSUM"))

        ident = pconst.tile([P, P], F32)
        make_identity(nc, ident)
        eps_t = pconst.tile([P, 1], F32)
        nc.vector.memset(eps_t, LN_EPS)
        # ones row at partition 64 (softmax-recip broadcast matmul lhsT)
        ones_bc = pconst.tile([P, P], F32R)
        nc.sync.dma_start(ones_bc[DK:DK + 1, :], dt_["cst_d"][0:1, :])

        def bcast_const(dst_ap, row):
            """DMA-broadcast cst row (0=ones, 1=zeros) into an arbitrary
            f32r SBUF access pattern: 0-stride outer dims, continuous inner."""
            n = dst_ap.shape[-1]
            src = dt_["cst_d"][row:row + 1, 0:n]
            ap = bass.AP(tensor=src.tensor, offset=src.offset,
                         ap=[[0, s] for s in dst_ap.shape[:-1]] + [[1, n]])
            nc.gpsimd.dma_start(dst_ap, ap)

        bias_sb = {}
        for nm in ("b1m", "bq", "bk", "pw1b", "dwb", "be1"):
            if fl[nm]:
                t = pconst.tile([P, bias_d[nm].shape[0] // P], F32)
                nc.sync.dma_start(t, bias_d[nm][:].rearrange("(a p) -> p a", p=P))
                bias_sb[nm] = t
        ones_row = None
        if any(fl[nm] for nm in ("b2m", "bo", "pw2b", "be2")):
            ones_row = pconst.tile([1, P], F32R)
            nc.sync.dma_start(ones_row, dt_["cst_d"][0:1, :])
        for nm in ("b2m", "bo", "pw2b", "be2"):
            if fl[nm]:
                t = pconst.tile([1, D], F32R)
                nc.sync.dma_start(t, bias_d[nm][:, :])
                bias_sb[nm] = t
        if fl["bv"]:
            src = bias_d["bv"][:]
            t = pconst.tile([P, D], F32)
            nc.gpsimd.dma_start(t, bass.AP(tensor=src.tensor, offset=src.offset,
                                           ap=[[0, P], src.ap[0]]))
            bias_sb["bv"] = t
        if fl["mask"]:
            t = pconst.tile([P, TT], F32)
            nc.sync.dma_start(t, bias_d["maskbias"][:]
                              .rearrange("(a p) -> p a", p=P))
            bias_sb["maskbias"] = t
        if fl["fin"]:
            for nm in ("g_fin", "b_fin"):
                src = bias_d[nm][:]
                t = pconst.tile([P, D], F32)
                nc.gpsimd.dma_start(t, bass.AP(tensor=src.tensor,
                                               offset=src.offset,
                                               ap=[[0, P], src.ap[0]]))
                bias_sb[nm] = t

        x_sb = px.tile([P, TT, D], F32)
        for tt in range(TT):
            nc.sync.dma_start(x_sb[:, tt, :], dt_["x_d"][tt * P:(tt + 1) * P, :])

        # ------------- helpers
        def ln_tile(tt):
            mv = pstat.tile([P, 2], F32, tag="mv")
            st = pstat.tile([P, 6], F32, tag="st")
            nc.vector.bn_stats(out=st, in_=x_sb[:, tt, :])
            nc.vector.bn_aggr(out=mv, in_=st)
            rstd = pstat.tile([P, 1], F32, tag="rstd")
            nc.scalar.activation(out=rstd, in_=mv[:, 1:2], func=AF.Sqrt,
                                 bias=eps_t, scale=1.0)
            nc.vector.reciprocal(out=rstd, in_=rstd)
            o = pxlnb.tile([P, D], F32, tag="xlnb")
            nc.vector.tensor_scalar(out=o, in0=x_sb[:, tt, :],
                                    scalar1=mv[:, 0:1], scalar2=rstd,
                                    op0=OP.subtract, op1=OP.mult)
            return o, mv, rstd

        def block_input():
            """LN(x_sb) transposed to layout A: [P(d), DP, T] f32r."""
            xlna = pxlna.tile([P, DP, T], F32R, tag="xlna")
            for g in range(2):
                tiles = [ln_tile(g * 4 + u)[0] for u in range(4)]
                for j in range(DP):
                    pt = ps_tp.tile([P, 512], F32, tag="tp")
                    for u in range(4):
                        nc.tensor.transpose(pt[:, u * P:(u + 1) * P],
                                            tiles[u][:, j * P:(j + 1) * P],
                                            ident)
                    nc.vector.tensor_copy(
                        out=xlna[:, j, g * 512:(g + 1) * 512], in_=pt)
            return xlna

        def mm_acc(pm, lhs_rhs, bias_nm):
            n = len(lhs_rhs)
            has_b = bias_nm is not None and fl.get(bias_nm, False)
            for i, (lh, rh) in enumerate(lhs_rhs):
                nc.tensor.matmul(pm, lh, rh, start=(i == 0),
                                 stop=(i == n - 1 and not has_b))
            if has_b:
                nc.tensor.matmul(pm, ones_row, bias_sb[bias_nm],
                                 start=False, stop=True)

        def ffn(xlna, w1_sb, w2_sb, b1nm, b2nm, ph1, ps_big, ps_mm):
            for c in range(2):
                h1 = ph1.tile([P, HP, 512], F32R, tag="h1")
                for hpp in range(0, HP, 2):
                    big = ps_big.tile([P, 1024], F32, tag="big")
                    for u in range(2):
                        hp = hpp + u
                        mm_acc(big[:, u * 512:(u + 1) * 512],
                               [(w1_sb[:, kd, hp * P:(hp + 1) * P],
                                 xlna[:, kd, c * 512:(c + 1) * 512])
                                for kd in range(DP)], None)
                    if fl[b1nm]:
                        for u in range(2):
                            nc.scalar.activation(
                                out=h1[:, hpp + u, :],
                                in_=big[:, u * 512:(u + 1) * 512], func=AF.Relu,
                                bias=bias_sb[b1nm][:, hpp + u:hpp + u + 1],
                                scale=1.0)
                    else:
                        nc.scalar.activation(out=h1[:, hpp:hpp + 2, :], in_=big,
                                             func=AF.Relu, scale=1.0)
                for j in range(4):
                    tt = c * 4 + j
                    pm = ps_mm.tile([P, 512], F32, tag="mm")
                    mm_acc(pm, [(h1[:, kh, j * P:(j + 1) * P], w2_sb[:, kh, :])
                                for kh in range(HP)], b2nm)
                    nc.vector.tensor_add(out=x_sb[:, tt, :], in0=pm,
                                         in1=x_sb[:, tt, :])

        # ================= phase 1: macaron FF =================
        with tc.tile_pool(name="pwmac", bufs=2) as pwmac, \
             tc.tile_pool(name="ph1a", bufs=1) as ph1, \
             tc.tile_pool(name="psbig1", bufs=2, space="PSUM") as ps_big, \
             tc.tile_pool(name="psmm1", bufs=2, space="PSUM") as ps_mm:
            w1_sb = pwmac.tile([P, DP, H], F32R, tag="wdh")
            for kd in range(DP):
                nc.sync.dma_start(w1_sb[:, kd, :],
                                  dt_["w1m_d"][kd * P:(kd + 1) * P, :])
            w2_sb = pwmac.tile([P, HP, D], F32R, tag="whd")
            for kh in range(HP):
                nc.sync.dma_start(w2_sb[:, kh, :],
                                  dt_["w2m_d"][kh * P:(kh + 1) * P, :])
            xlna = block_input()
            ffn(xlna, w1_sb, w2_sb, "b1m", "b2m", ph1, ps_big, ps_mm)

        # ================= phase 2: attention =================
        with tc.tile_pool(name="pqk", bufs=2) as pqk, \
             tc.tile_pool(name="pvaug", bufs=1) as pvaug, \
             tc.tile_pool(name="pexps", bufs=2) as pexps, \
             tc.tile_pool(name="pon", bufs=1) as pon, \
             tc.tile_pool(name="pwqkv", bufs=2) as pwqkv, \
             tc.tile_pool(name="pden", bufs=4) as pden, \
             tc.tile_pool(name="psbig2", bufs=2, space="PSUM") as ps_big, \
             tc.tile_pool(name="psmm2", bufs=2, space="PSUM") as ps_mm:

            xlna = block_input()

            def dd_weight(dram):
                w = pwqkv.tile([P, DP, D], F32R, tag="wdd")
                for kd in range(DP):
                    nc.sync.dma_start(w[:, kd, :], dram[kd * P:(kd + 1) * P, :])
                return w

            qk = {}
            for nm, dram, bnm in (("q", dt_["wq_d"], "bq"),
                                  ("k", dt_["wk_d"], "bk")):
                w = dd_weight(dram)
                acc = pqk.tile([P, DP, T], F32R, tag="qk")
                for dp in range(DP):
                    for c in range(2):
                        pm = ps_mm.tile([P, 512], F32, tag="mm")
                        mm_acc(pm, [(w[:, kd, dp * P:(dp + 1) * P],
                                     xlna[:, kd, c * 512:(c + 1) * 512])
                                    for kd in range(DP)], None)
                        if fl[bnm]:
                            nc.vector.tensor_scalar_add(
                                out=acc[:, dp, c * 512:(c + 1) * 512], in0=pm,
                                scalar1=bias_sb[bnm][:, dp:dp + 1])
                        else:
                            nc.vector.tensor_copy(
                                out=acc[:, dp, c * 512:(c + 1) * 512], in_=pm)
                qk[nm] = acc
            q_a, k_a = qk["q"], qk["k"]

            wv_sb = dd_weight(dt_["wv_d"])
            v_aug = pvaug.tile([P, TT, NH, DK + 1], F32R, tag="vaug")
            bcast_const(
                v_aug.rearrange("p a b c -> p (a b) c")[:, :, DK:DK + 1], 0)
            for tt in range(TT):
                pm = ps_mm.tile([P, 512], F32, tag="mm")
                mm_acc(pm, [(xlna[:, kd, tt * P:(tt + 1) * P], wv_sb[:, kd, :])
                            for kd in range(DP)], None)
                src = pm
                if fl["bv"]:
                    tmp = pden.tile([P, 512], F32, tag="vtmp")
                    nc.vector.tensor_add(out=tmp, in0=pm, in1=bias_sb["bv"])
                    src = tmp
                nc.vector.tensor_copy(
                    out=v_aug[:, tt, :, 0:DK],
                    in_=src[:, :].rearrange("p (h d) -> p h d", h=NH))

            wo_sb = dd_weight(dt_["wo_d"])
            o_n = pon.tile([P, DP, T], F32R, tag="on")

            for h in range(NH):
                hp, hr = h // 2, (h % 2) * DK
                for c in range(2):
                    exps = pexps.tile([P, TT, 512], F32R, tag="exps")
                    for g in range(4):
                        big = ps_big.tile([P, 1024], F32, tag="big")
                        for u in range(2):
                            tt = g * 2 + u
                            nc.tensor.matmul(
                                big[:, u * 512:(u + 1) * 512],
                                k_a[hr:hr + DK, hp, tt * P:(tt + 1) * P],
                                q_a[hr:hr + DK, hp, c * 512:(c + 1) * 512],
                                start=True, stop=True)
                        if fl["mask"]:
                            for u in range(2):
                                tt = g * 2 + u
                                nc.scalar.activation(
                                    out=exps[:, tt, :],
                                    in_=big[:, u * 512:(u + 1) * 512],
                                    func=AF.Exp,
                                    bias=bias_sb["maskbias"][:, tt:tt + 1],
                                    scale=1.0)
                        else:
                            nc.scalar.activation(out=exps[:, g * 2:g * 2 + 2, :],
                                                 in_=big, func=AF.Exp, scale=1.0)
                    pm = ps_mm.tile([P, 512], F32, tag="mm")
                    for tt in range(TT):
                        nc.tensor.matmul(pm[0:DK + 1, :], v_aug[:, tt, h, :],
                                         exps[:, tt, :],
                                         start=(tt == 0), stop=(tt == TT - 1))
                    # denominator -> recip at partition 64, broadcast via K=1 mm
                    dscr = pden.tile([P, 512], F32, tag="dscr")
                    nc.vector.tensor_copy(out=dscr[DK:DK + 1, :],
                                          in_=pm[DK:DK + 1, :])
                    rscr = pden.tile([P, 512], F32R, tag="rscr")
                    with nc.allow_low_precision(
                            reason="float32r rounding of softmax recip is the "
                                   "chosen matmul input precision"):
                        nc.vector.reciprocal(out=rscr[DK:DK + 1, :],
                                             in_=dscr[DK:DK + 1, :])
                    pb = ps_tp.tile([P, 512], F32, tag="tp")
                    nc.tensor.matmul(pb, ones_bc[DK:DK + 1, :],
                                     rscr[DK:DK + 1, :], start=True, stop=True)
                    bscr = pden.tile([DK, 512], F32, tag="bscr")
                    nc.scalar.activation(out=bscr, in_=pb[0:DK, :],
                                         func=AF.Copy, scale=1.0)
                    if hr == 0:
                        nc.vector.tensor_mul(
                            out=o_n[0:DK, hp, c * 512:(c + 1) * 512],
                            in0=pm[0:DK, :], in1=bscr)
                    else:
                        oscr = pden.tile([DK, 512], F32R, tag="oscr")
                        nc.vector.tensor_mul(out=oscr, in0=pm[0:DK, :],
                                             in1=bscr)
                        nc.sync.dma_start(
                            o_n[DK:P, hp, c * 512:(c + 1) * 512], oscr)

            for tt in range(TT):
                pm = ps_mm.tile([P, 512], F32, tag="mm")
                mm_acc(pm, [(o_n[:, kp, tt * P:(tt + 1) * P], wo_sb[:, kp, :])
                            for kp in range(DP)], "bo")
                nc.vector.tensor_add(out=x_sb[:, tt, :], in0=pm,
                                     in1=x_sb[:, tt, :])

        # ================= phases 3+4: conv module, then MoE FF ============
        with tc.tile_pool(name="pwe1", bufs=1) as pwe1:
            we1_sb = pwe1.tile([P, DP, H], F32R, tag="wdh")
            for kd in range(DP):
                nc.sync.dma_start(we1_sb[:, kd, :],
                                  dt_["we1_d"][kd * P:(kd + 1) * P, :])

            with tc.tile_pool(name="psig", bufs=1) as psig, \
                 tc.tile_pool(name="pglu", bufs=1) as pglu, \
                 tc.tile_pool(name="pswish", bufs=1) as pswish, \
                 tc.tile_pool(name="pcw", bufs=2) as pcw, \
                 tc.tile_pool(name="psgt", bufs=2) as psgt, \
                 tc.tile_pool(name="psbig3", bufs=2, space="PSUM") as ps_big, \
                 tc.tile_pool(name="psmm3", bufs=2, space="PSUM") as ps_mm:

                xlna = block_input()
                pw1_sb = pcw.tile([P, DP, 2 * D], F32R, tag="cw")
                for kd in range(DP):
                    nc.sync.dma_start(pw1_sb[:, kd, :],
                                      dt_["pw1_d"][kd * P:(kd + 1) * P, :])
                dww_sb = pconst.tile([P, DP, KTAPS], F32)
                nc.sync.dma_start(
                    dww_sb, dt_["dww_d"][:, :].rearrange("(a p) k -> p a k", p=P))

                sig_sb = psig.tile([P, DP, T], F32, tag="sig")
                for dpp in range(0, DP, 2):
                    for c in range(2):
                        big = ps_big.tile([P, 1024], F32, tag="big")
                        for u in range(2):
                            dp = dpp + u
                            mm_acc(big[:, u * 512:(u + 1) * 512],
                                   [(pw1_sb[:, kd, D + dp * P:D + (dp + 1) * P],
                                     xlna[:, kd, c * 512:(c + 1) * 512])
                                    for kd in range(DP)], None)
                        if fl["pw1b"]:
                            for u in range(2):
                                dp = dpp + u
                                nc.scalar.activation(
                                    out=sig_sb[:, dp, c * 512:(c + 1) * 512],
                                    in_=big[:, u * 512:(u + 1) * 512],
                                    func=AF.Sigmoid,
                                    bias=bias_sb["pw1b"][:, DP + dp:DP + dp + 1],
                                    scale=1.0)
                        else:
                            nc.scalar.activation(
                                out=sig_sb[:, dpp:dpp + 2,
                                           c * 512:(c + 1) * 512],
                                in_=big, func=AF.Sigmoid, scale=1.0)

                glu = pglu.tile([P, DP, T + 2 * PAD], F32R, tag="glu")
                bcast_const(glu[:, :, 0:PAD], 1)
                bcast_const(glu[:, :, PAD + T:], 1)
                for dpp in range(0, DP, 2):
                    for c in range(2):
                        big = ps_big.tile([P, 1024], F32, tag="big")
                        for u in range(2):
                            dp = dpp + u
                            mm_acc(big[:, u * 512:(u + 1) * 512],
                                   [(pw1_sb[:, kd, dp * P:(dp + 1) * P],
                                     xlna[:, kd, c * 512:(c + 1) * 512])
                                    for kd in range(DP)], None)
                        for u in range(2):
                            dp = dpp + u
                            src = big[:, u * 512:(u + 1) * 512]
                            if fl["pw1b"]:
                                tmp = psgt.tile([P, 512], F32, tag="sgt")
                                nc.vector.tensor_scalar_add(
                                    out=tmp, in0=src,
                                    scalar1=bias_sb["pw1b"][:, dp:dp + 1])
                                src = tmp
                            nc.vector.tensor_mul(
                                out=glu[:, dp,
                                        PAD + c * 512:PAD + (c + 1) * 512],
                                in0=src,
                                in1=sig_sb[:, dp, c * 512:(c + 1) * 512])

                swish = pswish.tile([P, DP, T], F32R, tag="swish")
                for dp in range(DP):
                    diag = pcw.tile([P, KTAPS, P], F32R, tag="cw")
                    for k in range(KTAPS):
                        nc.vector.tensor_scalar_mul(
                            out=diag[:, k, :], in0=ident,
                            scalar1=dww_sb[:, dp, k:k + 1])
                    for c in range(2):
                        pm = ps_mm.tile([P, 512], F32, tag="mm")
                        for k in range(KTAPS):
                            nc.tensor.matmul(
                                pm, diag[:, k, :],
                                glu[:, dp, k + c * 512:k + c * 512 + 512],
                                start=(k == 0), stop=(k == KTAPS - 1))
                        sgt = psgt.tile([P, 512], F32, tag="sgt")
                        if fl["dwb"]:
                            bcol = bias_sb["dwb"][:, dp:dp + 1]
                            nc.scalar.activation(out=sgt, in_=pm,
                                                 func=AF.Sigmoid, bias=bcol,
                                                 scale=1.0)
                            tmp2 = psgt.tile([P, 512], F32, tag="sgt")
                            nc.vector.tensor_scalar_add(out=tmp2, in0=pm,
                                                        scalar1=bcol)
                            nc.vector.tensor_mul(
                                out=swish[:, dp, c * 512:(c + 1) * 512],
                                in0=tmp2, in1=sgt)
                        else:
                            nc.scalar.activation(out=sgt, in_=pm,
                                                 func=AF.Sigmoid, scale=1.0)
                            nc.vector.tensor_mul(
                                out=swish[:, dp, c * 512:(c + 1) * 512],
                                in0=pm, in1=sgt)

                pw2_sb = pcw.tile([P, DP, D], F32R, tag="cw")
                for kd in range(DP):
                    nc.sync.dma_start(pw2_sb[:, kd, :],
                                      dt_["pw2_d"][kd * P:(kd + 1) * P, :])
                for tt in range(TT):
                    pm = ps_mm.tile([P, 512], F32, tag="mm")
                    mm_acc(pm, [(swish[:, kp, tt * P:(tt + 1) * P],
                                 pw2_sb[:, kp, :]) for kp in range(DP)], "pw2b")
                    nc.vector.tensor_add(out=x_sb[:, tt, :], in0=pm,
                                         in1=x_sb[:, tt, :])

            # -------- phase 4: MoE FF
            with tc.tile_pool(name="pwe2", bufs=1) as pwe2, \
                 tc.tile_pool(name="ph1b", bufs=1) as ph1, \
                 tc.tile_pool(name="psbig4", bufs=2, space="PSUM") as ps_big, \
                 tc.tile_pool(name="psmm4", bufs=2, space="PSUM") as ps_mm:
                we2_sb = pwe2.tile([P, HP, D], F32R, tag="whd")
                for kh in range(HP):
                    nc.sync.dma_start(we2_sb[:, kh, :],
                                      dt_["we2_d"][kh * P:(kh + 1) * P, :])
                xlna = block_input()
                ffn(xlna, we1_sb, we2_sb, "be1", "be2", ph1, ps_big, ps_mm)

        # ================= phase 5: final LN =================
        for tt in range(TT):
            o, mv, rstd = ln_tile(tt)
            if fl["fin"]:
                nc.vector.tensor_mul(out=o, in0=o, in1=bias_sb["g_fin"])
                nc.vector.tensor_add(out=o, in0=o, in1=bias_sb["b_fin"])
            nc.sync.dma_start(dt_["y_d"][tt * P:(tt + 1) * P, :], o)


# ---------------------------------------------------------------- entry

def kernel(**inputs) -> np.ndarray:
    fw = _fold_weights(inputs)
    fl = _flags(fw)
    key = tuple(sorted(fl.items()))
    if key not in _cache:
        _cache[key] = _build(fl)
    nc = _cache[key]

    cst = np.zeros((2, P), np.float32)
    cst[0, :] = 1.0
    shared = dict(
        w1m=fw["w1m"], w2m=fw["w2m"], wq=fw["wq"], wk=fw["wk"], wv=fw["wv"],
        wo=fw["wo"], pw1=fw["pw1"], pw2=fw["pw2"], dww=fw["dww"], cst=cst,
    )
    for nm in ("b1m", "bq", "bk", "bv", "pw1b", "dwb"):
        if fl[nm]:
            shared[nm] = fw[nm]
    for nm in ("b2m", "bo", "pw2b"):
        if fl[nm]:
            shared[nm] = fw[nm][None, :]
    if fl["fin"]:
        shared["g_fin"] = fw["g_fin"]
        shared["b_fin"] = fw["b_fin"]

    in_maps = []
    for b in range(NCORES):
        m = dict(shared)
        m["x"] = fw["x"][b]
        m["we1"] = fw["we1"][b]
        m["we2"] = fw["we2"][b]
        if fl["be1"]:
            m["be1"] = fw["be1"][b]
        if fl["be2"]:
            m["be2"] = fw["be2"][b][None, :]
        if fl["mask"]:
            m["maskbias"] = fw["maskbias"][b]
        in_maps.append(m)

    res = run_bass_kernel_spmd(nc, in_maps, core_ids=list(range(NCORES)))
    global last_result
    last_result = res
    out = np.stack([res.results[b]["y"] for b in range(NCORES)])
    return out.astype(np.float32)
